# revision 1
# baseline (speedup 1.0000x reference)
"""Trainium2 Bass kernel for nn_CocoaLoss (masked contrastive pair loss).

reference semantics:
    neg[i]  = (#zeros in label row i) > 1
    mask    = neg[:, None] & ~neg[None, :]
    count   = sum(mask)
    s(pred) = sum_{mask} exp(cos_sim(pred_i, pred_j) / 0.1)
    out     = LAM * (s(x)/count + s(y)/count)   (0 when count == 0)

Strategy (8 NeuronCores, data parallel over i-rows, 1024 rows/core):
  * every core loads the full labels, computes per-row neg flags and the
    global count on-device;
  * a device-side If(count > 0) guards the heavy phase entirely (with
    uniform 0/1 labels count is 0 with overwhelming probability, making
    the kernel memory-bound on the label read);
  * heavy phase: rows are L2-normalized, transposed via the PE into a
    [65, 8192] matrix whose extra row carries the column mask (-BIG for
    neg columns, 1s row on the lhsT side), so one K=65 matmul yields
    sim + colmask; exp(10*x + row_bias) runs on ACT with accum_out
    producing masked row sums directly;
  * partials are AllReduced across the 8 cores and the final scalar is
    computed on-device; core 0's output is returned.
"""

import numpy as np

import concourse.bacc as bacc
import concourse.bass as bass
import concourse.mybir as mybir
import concourse.tile as tile
from concourse import masks
from concourse.bass_utils import run_bass_kernel_spmd

B = 8192
D = 64
L = 32
NCORES = 8
ROWS_PER_CORE = B // NCORES  # 1024
ITILES_PER_CORE = ROWS_PER_CORE // 128  # 8
NTILES = B // 128  # 64
TAU = 0.1
LAM = 1.0
THRESH_SUM = L - 2  # neg  <=>  zeros > 1  <=>  sum(labels) <= 30
BIG = 50000.0
MM_N = 512  # matmul moving free dim (fp32 max)
CHUNK = 2048  # psum chunk (4 banks); 4 chunks cover the 8192 columns
NCHUNKS = B // CHUNK  # 4

F32 = mybir.dt.float32
I32 = mybir.dt.int32

_CACHE: dict = {}
LAST_RESULT = None  # BassKernelResults of the most recent run (for test.py)


def _build(w: int, with_collective: bool = True) -> bass.Bass:
    """Build the SPMD program. `w` = int32 words per label row (32 when the
    labels arrive int32, 64 when int64 viewed as int32 pairs; the odd high
    words of small nonnegative int64 are 0 so a plain row-sum works).
    with_collective=False swaps the AllReduce for a local copy so the
    single-core timeline simulator can run the program."""
    nc = bacc.Bacc(
        "TRN2", target_bir_lowering=False, debug=False, num_devices=NCORES
    )

    xt = nc.dram_tensor("x_full", [B, D], F32, kind="ExternalInput")
    yt = nc.dram_tensor("y_full", [B, D], F32, kind="ExternalInput")
    lab = nc.dram_tensor("lab_full", [B, w], I32, kind="ExternalInput")
    out = nc.dram_tensor("out", [1, 1], F32, kind="ExternalOutput")

    with tile.TileContext(nc) as tc:
        with (
            tc.tile_pool(name="const", bufs=1) as cpool,
            tc.tile_pool(name="labp", bufs=1) as labp,
            tc.tile_pool(name="inbuf", bufs=2) as inbuf,
            tc.tile_pool(name="pnp", bufs=2) as pnp,
            tc.tile_pool(name="work", bufs=3) as work,
            tc.tile_pool(name="mmps", bufs=2, space="PSUM") as mmps,
            tc.tile_pool(name="dram", bufs=2, space="DRAM") as dram,
        ):
            ident = cpool.tile([128, 128], F32)
            masks.make_identity(nc, ident[:])
            ones128 = cpool.tile([128, 1], F32)
            nc.vector.memset(ones128[:], 1.0)

            # ---- phase 0: labels -> neg flags + count (always runs) ----
            labt = labp.tile([128, NTILES * w], I32)
            lab_r = lab.rearrange("(t p) w -> p t w", p=128)
            labt_3 = labt[:].rearrange("p (t w) -> p t w", w=w)
            for g in range(8):  # 8 DMAs so several queues run in parallel
                sl = slice(g * 8, (g + 1) * 8)
                nc.sync.dma_start(labt_3[:, sl, :], lab_r[:, sl, :])

            # reduce per DMA chunk: one sem wait per instruction (HW limit)
            lsums = cpool.tile([128, NTILES], I32)
            with nc.allow_low_precision(reason="int32 label sums <= 32 are exact"):
                for g in range(8):
                    sl = slice(g * 8, (g + 1) * 8)
                    nc.vector.reduce_sum(
                        lsums[:, sl], labt_3[:, sl, :], axis=mybir.AxisListType.X
                    )
            # negs[p, t] = 1.0 if row 128*t+p is negative else 0.0
            negs = cpool.tile([128, NTILES], F32)
            nc.vector.tensor_scalar(
                negs[:], lsums[:], THRESH_SUM, None, mybir.AluOpType.is_le
            )

            # count = n_neg * (B - n_neg), exact in f32 (<= 2^24)
            nneg_ps = mmps.tile([1, NTILES], F32, tag="mm")
            nc.tensor.matmul(nneg_ps[:], ones128[:], negs[:], start=True, stop=True)
            nneg = cpool.tile([1, 1], F32)
            nc.vector.reduce_sum(nneg[:], nneg_ps[:], axis=mybir.AxisListType.X)
            npos = cpool.tile([1, 1], F32)
            nc.vector.tensor_scalar(
                npos[:], nneg[:], -1.0, float(B), mybir.AluOpType.mult,
                mybir.AluOpType.add,
            )
            count = cpool.tile([1, 1], F32)
            nc.vector.tensor_mul(count[:], nneg[:], npos[:])

            pid = nc.partition_id()

            # per-(input, i-tile, chunk) masked row sums land here (written
            # only in the count>0 branch, and read only there)
            acc = cpool.tile([128, 2 * ITILES_PER_CORE * NCHUNKS], F32)

            # final per-core result; stays 0 when count == 0
            res = cpool.tile([1, 1], F32)
            nc.vector.memset(res[:], 0.0)

            cnt_bits = nc.values_load(
                count[0:1, 0:1].bitcast(I32).to_broadcast((1, 1))
            )

            # ---- heavy phase + AllReduce, skipped entirely when count == 0.
            # count is computed from the full labels identically on every
            # core, so the branch decision is uniform across ranks and the
            # collective either runs on all 8 ranks or on none. ----
            with tc.If(cnt_bits > 0, preferred_fallthrough_block=False):
                # factor = LAM / count (count > 0 in this branch)
                factor = cpool.tile([1, 1], F32)
                nc.vector.reciprocal(factor[:], count[:])
                if LAM != 1.0:
                    nc.vector.tensor_scalar_mul(factor[:], factor[:], LAM)
                # row-mask bias for this core's 8 i-tiles: 0 if neg else -BIG
                # (dynamic pid-based read on DVE; ACT bias slices stay static)
                bias8 = cpool.tile([128, ITILES_PER_CORE], F32)
                for k in range(ITILES_PER_CORE):
                    nc.vector.tensor_scalar(
                        bias8[:, k : k + 1],
                        negs[:, bass.ds(pid * ITILES_PER_CORE + k, 1)],
                        BIG, -BIG, mybir.AluOpType.mult, mybir.AluOpType.add,
                    )
                for inp_idx, src in enumerate((xt, yt)):
                    # load all rows [128, 64*64]; tile t = rows 128t..128t+127
                    xbuf = inbuf.tile([128, NTILES * D], F32, tag="xin")
                    src_r = src.rearrange("(t p) d -> p t d", p=128)
                    xbuf_3 = xbuf[:].rearrange("p (t d) -> p t d", d=D)
                    for g in range(8):
                        sl = slice(g * 8, (g + 1) * 8)
                        nc.sync.dma_start(xbuf_3[:, sl, :], src_r[:, sl, :])

                    # row norms, sliced per DMA chunk (one sem wait each)
                    sq = inbuf.tile([128, NTILES * D], F32, tag="sq")
                    ss = work.tile([128, NTILES], F32, tag="ss")
                    sq_3 = sq[:].rearrange("p (t d) -> p t d", d=D)
                    for g in range(8):
                        c = slice(g * 8 * D, (g + 1) * 8 * D)
                        nc.vector.tensor_mul(sq[:, c], xbuf[:, c], xbuf[:, c])
                        nc.vector.reduce_sum(
                            ss[:, g * 8 : (g + 1) * 8],
                            sq_3[:, g * 8 : (g + 1) * 8, :],
                            axis=mybir.AxisListType.X,
                        )
                    # 1/||row|| = exp(-0.5*ln(ss)): Log and Exp share one ACT
                    # table set, avoiding sqrt<->exp table switches
                    lnss = work.tile([128, NTILES], F32, tag="nrm")
                    nc.scalar.activation(
                        lnss[:], ss[:], mybir.ActivationFunctionType.Ln
                    )
                    rn = work.tile([128, NTILES], F32, tag="rn")
                    nc.scalar.activation(
                        rn[:], lnss[:], mybir.ActivationFunctionType.Exp, scale=-0.5
                    )

                    # pnr[0:64, j] = normalized row j (transposed);
                    # pnr[64, j]   = -BIG if j negative else 0  (column mask)
                    pnr = pnp.tile([65, B], F32, tag="pnr")
                    for t in range(NTILES):
                        aug = work.tile([128, 65], F32, tag="aug")
                        nc.vector.tensor_scalar_mul(
                            aug[:, 0:D],
                            xbuf[:, t * D : (t + 1) * D],
                            rn[:, t : t + 1],
                        )
                        nc.vector.tensor_scalar_mul(
                            aug[:, D : D + 1], negs[:, t : t + 1], -BIG
                        )
                        tps = mmps.tile([65, 128], F32, tag="mm")
                        nc.tensor.transpose(tps[:], aug[:], ident[:])
                        nc.vector.tensor_copy(
                            pnr[:, t * 128 : (t + 1) * 128], tps[:]
                        )

                    # lhsT source: this core's 1024 columns, ones in row 64
                    fm = pnp.tile([65, ROWS_PER_CORE], F32, tag="fm")
                    nc.vector.tensor_copy(
                        fm[0:64, :],
                        pnr[0:64, bass.ds(pid * ROWS_PER_CORE, ROWS_PER_CORE)],
                    )
                    nc.vector.memset(fm[64:65, :], 1.0)

                    for k in range(ITILES_PER_CORE):
                        lhsT = fm[:, k * 128 : (k + 1) * 128]
                        for m in range(NCHUNKS):
                            ps = mmps.tile([128, CHUNK], F32, tag="mm")
                            for n in range(CHUNK // MM_N):
                                c0 = m * CHUNK + n * MM_N
                                nc.tensor.matmul(
                                    ps[:, n * MM_N : (n + 1) * MM_N],
                                    lhsT,
                                    pnr[:, c0 : c0 + MM_N],
                                    start=True,
                                    stop=True,
                                )
                            # exp in place in PSUM (ScE->PSUM is the fast port;
                            # the tile is dead after the accumulated row sums)
                            col = (inp_idx * ITILES_PER_CORE + k) * NCHUNKS + m
                            nc.scalar.activation(
                                ps[:],
                                ps[:],
                                mybir.ActivationFunctionType.Exp,
                                bias=bias8[:, k : k + 1],
                                scale=1.0 / TAU,
                                accum_out=acc[:, col : col + 1],
                            )

                # c_core = factor * sum(acc); AllReduce of c_core IS the
                # answer (factor is identical on every core; sum is linear)
                accsum = cpool.tile([128, 1], F32)
                nc.vector.reduce_sum(accsum[:], acc[:], axis=mybir.AxisListType.X)
                part_ps = mmps.tile([1, 1], F32, tag="mm")
                nc.tensor.matmul(
                    part_ps[:], ones128[:], accsum[:], start=True, stop=True
                )
                cpart = cpool.tile([1, 1], F32)
                nc.vector.tensor_scalar(
                    cpart[:], part_ps[:], factor[0:1, 0:1], None,
                    mybir.AluOpType.mult,
                )

                cc_in = dram.tile([1, 1], F32)
                cc_out = dram.tile([1, 1], F32)
                nc.sync.dma_start(cc_in[:], cpart[:])
                if with_collective:
                    nc.gpsimd.collective_compute(
                        "AllReduce",
                        mybir.AluOpType.add,
                        replica_groups=[list(range(NCORES))],
                        ins=[cc_in.opt()],
                        outs=[cc_out.opt()],
                    )
                else:
                    nc.sync.dma_start(cc_out[:], cc_in[:])
                nc.sync.dma_start(res[:], cc_out[:])

            # ---- always runs ----
            nc.sync.dma_start(out[0:1, 0:1], res[:])

    nc.compile()
    return nc


def _labels_as_i32(lab: np.ndarray) -> tuple[np.ndarray, int]:
    lab = np.ascontiguousarray(np.asarray(lab))
    if lab.dtype == np.int64:
        return lab.view(np.int32).reshape(B, 2 * L), 2 * L
    if lab.dtype == np.int32:
        return lab, L
    return np.ascontiguousarray(lab.astype(np.int32)), L


def kernel(**inputs) -> np.ndarray:
    global LAST_RESULT
    x = np.ascontiguousarray(np.asarray(inputs["x_pred_batch"], dtype=np.float32))
    y = np.ascontiguousarray(np.asarray(inputs["y_pred_batch"], dtype=np.float32))
    labi, w = _labels_as_i32(inputs["label_batch"])
    assert x.shape == (B, D) and y.shape == (B, D)

    if w not in _CACHE:
        _CACHE[w] = _build(w)
    nc = _CACHE[w]

    in_map = {"x_full": x, "y_full": y, "lab_full": labi}
    LAST_RESULT = run_bass_kernel_spmd(
        nc, [dict(in_map) for _ in range(NCORES)], core_ids=list(range(NCORES))
    )
    return np.asarray(
        LAST_RESULT.results[0]["out"], dtype=np.float32
    ).reshape(())


if __name__ == "__main__":
    rng = np.random.default_rng(0)
    xs = rng.standard_normal((B, D)).astype(np.float32)
    ys = rng.standard_normal((B, D)).astype(np.float32)
    ls = (rng.random((B, L)) > 0.5).astype(np.int64)
    print(kernel(x_pred_batch=xs, y_pred_batch=ys, label_batch=ls))



# revision 8
# speedup vs baseline: 1.7214x; 1.7214x over previous
"""Trainium2 Bass kernel for nn_CocoaLoss (masked contrastive pair loss).

reference semantics:
    neg[i]  = (#zeros in label row i) > 1
    mask    = neg[:, None] & ~neg[None, :]
    count   = sum(mask)
    s(pred) = sum_{mask} exp(cos_sim(pred_i, pred_j) / 0.1)
    out     = LAM * (s(x)/count + s(y)/count)   (0 when count == 0)

Strategy (8 NeuronCores, data parallel over i-rows, 1024 rows/core):
  * every core loads the full labels, computes per-row neg flags and the
    global count on-device;
  * a device-side If(count > 0) guards the heavy phase entirely (with
    uniform 0/1 labels count is 0 with overwhelming probability, making
    the kernel memory-bound on the label read);
  * the row order is globally permuted to r = p*64 + t (partition-major)
    so every DMA descriptor is one contiguous 2-8KB run per partition
    (128 descriptors/DMA instead of 8192); the loss is a sum over (i, j)
    pairs, so any consistent row permutation leaves it unchanged;
  * fast path: the label load is split into 4 size-tapered DMA chunks
    whose per-row sums run on DVE and Pool while later chunks are still
    in flight; out=0 is DMA'd early (overlapped) so the not-taken branch
    ends without an output-DMA tail;
  * heavy phase: rows are L2-normalized, transposed via the PE into a
    [65, 8192] matrix whose extra row carries the column mask (-BIG for
    neg columns, 1s row on the lhsT side), so one K=65 matmul yields
    sim + colmask; exp(10*x + row_bias) runs on ACT with accum_out
    producing masked row sums directly;
  * partials are AllReduced across the 8 cores and the final scalar is
    computed on-device; core 0's output is returned.
"""

import numpy as np

import concourse.bacc as bacc
import concourse.bass as bass
import concourse.mybir as mybir
import concourse.tile as tile
from concourse import masks
from concourse.bass_utils import run_bass_kernel_spmd

B = 8192
D = 64
L = 32
NCORES = 8
ROWS_PER_CORE = B // NCORES  # 1024
ITILES_PER_CORE = ROWS_PER_CORE // 128  # 8
NTILES = B // 128  # 64
TAU = 0.1
LAM = 1.0
THRESH_SUM = L - 2  # neg  <=>  zeros > 1  <=>  sum(labels) <= 30
BIG = 50000.0
MM_N = 512  # matmul moving free dim (fp32 max)
CHUNK = 2048  # psum chunk (4 banks); 4 chunks cover the 8192 columns
NCHUNKS = B // CHUNK  # 4

# fast-path label DMA chunking: tiles (rows) per partition per chunk.
# Tapered (brute-forced against the v2 cost model) so DVE row sums of
# chunk c overlap the transfers of chunks c+1.. and the last chunk's
# sem-prop + reduce tail is short.
LAB_CHUNKS = [19, 15, 12, 10, 8]

F32 = mybir.dt.float32
I32 = mybir.dt.int32

_CACHE: dict = {}
LAST_RESULT = None  # BassKernelResults of the most recent run (for test.py)


def _build(w: int, with_collective: bool = True) -> bass.Bass:
    """Build the SPMD program. `w` = int32 words per label row (32 when the
    labels arrive int32, 64 when int64 viewed as int32 pairs; the odd high
    words of small nonnegative int64 are 0 so a plain row-sum works).
    with_collective=False swaps the AllReduce for a local copy so the
    single-core timeline simulator can run the program."""
    nc = bacc.Bacc(
        "TRN2", target_bir_lowering=False, debug=False, num_devices=NCORES
    )

    xt = nc.dram_tensor("x_full", [B, D], F32, kind="ExternalInput")
    yt = nc.dram_tensor("y_full", [B, D], F32, kind="ExternalInput")
    lab = nc.dram_tensor("lab_full", [B, w], I32, kind="ExternalInput")
    out = nc.dram_tensor("out", [1, 1], F32, kind="ExternalOutput")

    with tile.TileContext(nc) as tc:
        with (
            tc.tile_pool(name="const", bufs=1) as cpool,
            tc.tile_pool(name="labp", bufs=1) as labp,
            tc.tile_pool(name="inbuf", bufs=2) as inbuf,
            tc.tile_pool(name="pnp", bufs=2) as pnp,
            tc.tile_pool(name="work", bufs=3) as work,
            tc.tile_pool(name="mmps", bufs=2, space="PSUM") as mmps,
            tc.tile_pool(name="dram", bufs=2, space="DRAM") as dram,
        ):
            # ---- t=0: result placeholder + early out=0 DMA setup ----
            res = cpool.tile([1, 1], F32)
            nc.vector.memset(res[:], 0.0)
            ones128 = cpool.tile([128, 1], F32)

            # ---- phase 0: labels -> neg flags + count (always runs) ----
            # Row r = p*64 + t lives on partition p, tile t: each partition's
            # 64 rows are contiguous in DRAM -> one descriptor per partition
            # per DMA. Chunked so per-chunk row sums overlap later transfers.
            labt = labp.tile([128, NTILES * w], I32)
            lab_r = lab.rearrange("(p t) w -> p t w", p=128)
            labt_3 = labt[:].rearrange("p (t w) -> p t w", w=w)
            bounds = np.cumsum([0] + LAB_CHUNKS)
            for c in range(len(LAB_CHUNKS)):
                sl = slice(bounds[c], bounds[c + 1])
                nc.sync.dma_start(labt_3[:, sl, :], lab_r[:, sl, :])
            # out starts as 0; issued after the label DMAs on SP so its HWDGE
            # slot never delays a label transfer. The heavy branch overwrites
            # it later through `res` (the WAR on res orders the two).
            nc.sync.dma_start(out[0:1, 0:1], res[:])

            # per-chunk row sums (free-axis reduce is DVE-only) while later
            # chunks are in flight.
            lsums = cpool.tile([128, NTILES], I32)
            with nc.allow_low_precision(reason="int32 label sums <= 32 are exact"):
                for c in range(len(LAB_CHUNKS)):
                    sl = slice(bounds[c], bounds[c + 1])
                    nc.vector.reduce_sum(
                        lsums[:, sl], labt_3[:, sl, :], axis=mybir.AxisListType.X
                    )
            # count > 0  <=>  (exists pos row) and (exists neg row), and a
            # pos row exists iff max row-sum > 30. With uniform 0/1 labels
            # ~no row is pos, so the fast path needs only ONE cross-engine
            # op after the row sums: a GpSimd all-axis max.
            lmax = cpool.tile([1, 1], I32)
            nc.gpsimd.reduce_max(lmax[:], lsums[:], axis=mybir.AxisListType.XYZWC)

            pid = nc.partition_id()

            # per-(input, i-tile, chunk) masked row sums land here (written
            # only in the taken branch, and read only there)
            acc = cpool.tile([128, 2 * ITILES_PER_CORE * NCHUNKS], F32)

            mx_bits = nc.values_load(lmax[0:1, 0:1].to_broadcast((1, 1)))

            # ---- heavy phase + AllReduce, skipped when no pos row exists.
            # lmax comes from the full labels identically on every core, so
            # the branch decision is uniform across ranks and the collective
            # either runs on all 8 ranks or on none. ----
            with tc.If(mx_bits > THRESH_SUM, preferred_fallthrough_block=False):
                # negs[p, t] = 1.0 if row p*64+t is negative else 0.0
                negs = cpool.tile([128, NTILES], F32)
                nc.vector.tensor_scalar(
                    negs[:], lsums[:], THRESH_SUM, None, mybir.AluOpType.is_le
                )
                # count = n_neg * (B - n_neg), exact in f32 (<= 2^24)
                nneg = cpool.tile([1, 1], F32)
                nc.gpsimd.reduce_sum(
                    nneg[:], negs[:], axis=mybir.AxisListType.XYZWC
                )
                npos = cpool.tile([1, 1], F32)
                nc.gpsimd.tensor_scalar(
                    npos[:], nneg[:], -1.0, float(B), mybir.AluOpType.mult,
                    mybir.AluOpType.add,
                )
                count = cpool.tile([1, 1], F32)
                nc.gpsimd.tensor_mul(count[:], nneg[:], npos[:])

                ident = cpool.tile([128, 128], F32)
                masks.make_identity(nc, ident[:])
                nc.vector.memset(ones128[:], 1.0)
                # factor = LAM / max(count, 1): count can only be 0 here in
                # the measure-zero "pos rows but no neg rows" case, where the
                # masked sums are 0 too, so clamping keeps the result exact
                # (0) without a nested If around the collective.
                cnt1 = cpool.tile([1, 1], F32)
                nc.gpsimd.tensor_scalar(
                    cnt1[:], count[:], 1.0, None, mybir.AluOpType.max
                )
                factor = cpool.tile([1, 1], F32)
                nc.vector.reciprocal(factor[:], cnt1[:])
                if LAM != 1.0:
                    nc.vector.tensor_scalar_mul(factor[:], factor[:], LAM)
                # row-mask bias for this core's 8 i-tiles: 0 if neg else -BIG
                # (dynamic pid-based read on DVE; ACT bias slices stay static)
                bias8 = cpool.tile([128, ITILES_PER_CORE], F32)
                for k in range(ITILES_PER_CORE):
                    nc.vector.tensor_scalar(
                        bias8[:, k : k + 1],
                        negs[:, bass.ds(pid * ITILES_PER_CORE + k, 1)],
                        BIG, -BIG, mybir.AluOpType.mult, mybir.AluOpType.add,
                    )
                for inp_idx, src in enumerate((xt, yt)):
                    # load all rows [128, 64*64]; tile t = rows p*64+t, same
                    # partition-major order as the labels (contiguous 2KB
                    # descriptors per partition per 8-tile group)
                    xbuf = inbuf.tile([128, NTILES * D], F32, tag="xin")
                    src_r = src.rearrange("(p t) d -> p t d", p=128)
                    xbuf_3 = xbuf[:].rearrange("p (t d) -> p t d", d=D)
                    for g in range(8):
                        sl = slice(g * 8, (g + 1) * 8)
                        nc.sync.dma_start(xbuf_3[:, sl, :], src_r[:, sl, :])

                    # row norms, sliced per DMA chunk (one sem wait each)
                    sq = inbuf.tile([128, NTILES * D], F32, tag="sq")
                    ss = work.tile([128, NTILES], F32, tag="ss")
                    sq_3 = sq[:].rearrange("p (t d) -> p t d", d=D)
                    for g in range(8):
                        c = slice(g * 8 * D, (g + 1) * 8 * D)
                        nc.vector.tensor_mul(sq[:, c], xbuf[:, c], xbuf[:, c])
                        nc.vector.reduce_sum(
                            ss[:, g * 8 : (g + 1) * 8],
                            sq_3[:, g * 8 : (g + 1) * 8, :],
                            axis=mybir.AxisListType.X,
                        )
                    # 1/||row|| = exp(-0.5*ln(ss)): Log and Exp share one ACT
                    # table set, avoiding sqrt<->exp table switches
                    lnss = work.tile([128, NTILES], F32, tag="nrm")
                    nc.scalar.activation(
                        lnss[:], ss[:], mybir.ActivationFunctionType.Ln
                    )
                    rn = work.tile([128, NTILES], F32, tag="rn")
                    nc.scalar.activation(
                        rn[:], lnss[:], mybir.ActivationFunctionType.Exp, scale=-0.5
                    )

                    # pnr[0:64, j] = normalized row rho(j) (transposed), where
                    # column j = t*128 + p holds row rho(j) = p*64 + t;
                    # pnr[64, j] = -BIG if rho(j) negative else 0 (column mask)
                    pnr = pnp.tile([65, B], F32, tag="pnr")
                    for t in range(NTILES):
                        aug = work.tile([128, 65], F32, tag="aug")
                        nc.vector.tensor_scalar_mul(
                            aug[:, 0:D],
                            xbuf[:, t * D : (t + 1) * D],
                            rn[:, t : t + 1],
                        )
                        nc.vector.tensor_scalar_mul(
                            aug[:, D : D + 1], negs[:, t : t + 1], -BIG
                        )
                        tps = mmps.tile([65, 128], F32, tag="mm")
                        nc.tensor.transpose(tps[:], aug[:], ident[:])
                        nc.vector.tensor_copy(
                            pnr[:, t * 128 : (t + 1) * 128], tps[:]
                        )

                    # lhsT source: this core's 1024 columns, ones in row 64.
                    # Column pid*1024 + k*128 + p of pnr is row p*64 + pid*8+k,
                    # so bias8[:, k] (= -BIG*negs[:, pid*8+k]) matches psum
                    # partition p of i-tile k exactly.
                    fm = pnp.tile([65, ROWS_PER_CORE], F32, tag="fm")
                    nc.vector.tensor_copy(
                        fm[0:64, :],
                        pnr[0:64, bass.ds(pid * ROWS_PER_CORE, ROWS_PER_CORE)],
                    )
                    nc.vector.memset(fm[64:65, :], 1.0)

                    for k in range(ITILES_PER_CORE):
                        lhsT = fm[:, k * 128 : (k + 1) * 128]
                        for m in range(NCHUNKS):
                            ps = mmps.tile([128, CHUNK], F32, tag="mm")
                            for n in range(CHUNK // MM_N):
                                c0 = m * CHUNK + n * MM_N
                                nc.tensor.matmul(
                                    ps[:, n * MM_N : (n + 1) * MM_N],
                                    lhsT,
                                    pnr[:, c0 : c0 + MM_N],
                                    start=True,
                                    stop=True,
                                )
                            # exp in place in PSUM (ScE->PSUM is the fast port;
                            # the tile is dead after the accumulated row sums)
                            col = (inp_idx * ITILES_PER_CORE + k) * NCHUNKS + m
                            nc.scalar.activation(
                                ps[:],
                                ps[:],
                                mybir.ActivationFunctionType.Exp,
                                bias=bias8[:, k : k + 1],
                                scale=1.0 / TAU,
                                accum_out=acc[:, col : col + 1],
                            )

                # c_core = factor * sum(acc); AllReduce of c_core IS the
                # answer (factor is identical on every core; sum is linear)
                accsum = cpool.tile([128, 1], F32)
                nc.vector.reduce_sum(accsum[:], acc[:], axis=mybir.AxisListType.X)
                part_ps = mmps.tile([1, 1], F32, tag="mm")
                nc.tensor.matmul(
                    part_ps[:], ones128[:], accsum[:], start=True, stop=True
                )
                cpart = cpool.tile([1, 1], F32)
                nc.vector.tensor_scalar(
                    cpart[:], part_ps[:], factor[0:1, 0:1], None,
                    mybir.AluOpType.mult,
                )

                cc_in = dram.tile([1, 1], F32)
                cc_out = dram.tile([1, 1], F32)
                nc.sync.dma_start(cc_in[:], cpart[:])
                if with_collective:
                    nc.gpsimd.collective_compute(
                        "AllReduce",
                        mybir.AluOpType.add,
                        replica_groups=[list(range(NCORES))],
                        ins=[cc_in.opt()],
                        outs=[cc_out.opt()],
                    )
                else:
                    nc.sync.dma_start(cc_out[:], cc_in[:])
                # res <- AllReduce result, then overwrite out (ordered after
                # the early out=0 DMA by the WAR dependency on res)
                nc.sync.dma_start(res[:], cc_out[:])
                nc.sync.dma_start(out[0:1, 0:1], res[:])

    nc.compile()
    return nc


def _labels_as_i32(lab: np.ndarray) -> tuple[np.ndarray, int]:
    lab = np.ascontiguousarray(np.asarray(lab))
    if lab.dtype == np.int64:
        return lab.view(np.int32).reshape(B, 2 * L), 2 * L
    if lab.dtype == np.int32:
        return lab, L
    return np.ascontiguousarray(lab.astype(np.int32)), L


def kernel(**inputs) -> np.ndarray:
    global LAST_RESULT
    x = np.ascontiguousarray(np.asarray(inputs["x_pred_batch"], dtype=np.float32))
    y = np.ascontiguousarray(np.asarray(inputs["y_pred_batch"], dtype=np.float32))
    labi, w = _labels_as_i32(inputs["label_batch"])
    assert x.shape == (B, D) and y.shape == (B, D)

    if w not in _CACHE:
        _CACHE[w] = _build(w)
    nc = _CACHE[w]

    in_map = {"x_full": x, "y_full": y, "lab_full": labi}
    LAST_RESULT = run_bass_kernel_spmd(
        nc, [dict(in_map) for _ in range(NCORES)], core_ids=list(range(NCORES))
    )
    return np.asarray(
        LAST_RESULT.results[0]["out"], dtype=np.float32
    ).reshape(())


if __name__ == "__main__":
    rng = np.random.default_rng(0)
    xs = rng.standard_normal((B, D)).astype(np.float32)
    ys = rng.standard_normal((B, D)).astype(np.float32)
    ls = (rng.random((B, L)) > 0.5).astype(np.int64)
    print(kernel(x_pred_batch=xs, y_pred_batch=ys, label_batch=ls))
